# revision 12
# baseline (speedup 1.0000x reference)
"""Trainium2 Bass kernel for FISTA sparse coding (nn_FISTA_7550552506950).

Strategy (data-parallel over batch, 8 cores x 128 rows; single-f32 state):
- State z kept TRANSPOSED [F=4096, B=128] on-chip as ONE f32 tensor
  [128p, 32 chunks, 256 cols] (real|imag column halves), SBUF-resident for
  all 25 FISTA iterations. HBM traffic is only the initial weight/x load and
  the final magnitude store.
- The z-carrying matmuls (momentum identities, A-chain) run with float32r
  operands (single-pass relaxed fp32; N>=256 keeps full rate): z precision
  dominates the error budget (fp16 z would cost 1.3e-2 vs the 2e-2 gate;
  f32r's ~12-bit ingest gives ~7e-3).
- The gradient matmuls run fully in fp16: the residual quantization is
  benign (sim: 7e-4). Rcat is pre-scaled by 64*step and W2 by 1/64 so all
  significant fp16 values stay in the normal range.
- A-chain: one N=256 matmul per chunk accumulates the quadrant products
  Pcat = [[Dr zr | Dr zi],[Di zr | Di zi]]  ([128, 256] PSUM).
- A fixed +-1 rotation matrix (partition swap t<->t+64 with sign flip,
  rot(v) = [v_hi; -v_lo]) applied by one N=256 matmul turns the quadrants
  into stacked complex products: P2 = [Re; Im](D z) = PcatL - rot(PcatR) and
  rot(P2) = rot(PcatL) + PcatR, from which GPSIMD+DVE build
  Rcat = -64*step*[resid | rot(resid)] with the FISTA momentum folded in by
  linearity (resid = a*P2 + b*P2_old - X; X terms are host-precomputed).
- Gradient: one N=256 fp16 matmul per chunk u[:,c,:] += W2cat_c^T @ Rcat
  yields both -step*Re(D^H resid) and -step*Im(D^H resid) in one go.
- The elementwise momentum u += a*z + b*z_old is folded into the PSUM
  accumulation via scaled-identity f32r matmuls (N=512), with a = fp16(1+gam)
  and b = 1-a exactly representable so the z-coefficient rounding cancels.
- Soft-threshold: t12 = u^2 (ACT), m2 = t12r+t12i (GPSIMD),
  st = rsqrt(m2 * 2^26) = thr*rsqrt(m2) (ACT raw path, thr folded into the
  activation scale - 1/thr^2 is exactly 2^26), then s = 1 - min(st, 1) via
  two DVE tensor_scalar ops (2x port mode) for most groups / ACT relu for
  the tail groups, z = u*s (DVE, one f32 pass).
- Software pipeline: grad lags momentum by 1 slot, the threshold tail by 2
  (PSUM u pool has 3 bufs = exactly the in-flight window), and the next
  iteration's A-chain follows each group's z-write by 1 slot. The leftover
  A-group + rotation + Rcat build at each iteration boundary hide behind the
  momentum matmuls of the first two slots.
- Global max normalization happens on host during the gather (tiny).
"""

import numpy as np
from contextlib import ExitStack

import concourse.bass as bass
import concourse.mybir as mybir
import concourse.tile as tile
from concourse import bacc
from concourse.bass_utils import run_bass_kernel_spmd

F32 = mybir.dt.float32
F32R = mybir.dt.float32r
FP16 = mybir.dt.float16
ALU = mybir.AluOpType
ACTF = mybir.ActivationFunctionType

P = 128          # partitions / f-chunk size
F = 4096         # dictionary size
T = 64           # signal dim
NCH = F // P     # 32 chunks
B = 128          # batch rows per core
NCORES = 8
MAX_ITER = 25
STEP = np.float32(1.0 / F)
THR = np.float32(0.5) * STEP
SC = np.float32(64.0) * STEP   # fp16 scaling for the gradient path (2^-6)
RSQ_SCALE = float(1.0 / (THR * THR))   # exactly 2^26
GRP = 4          # chunks per elementwise group
NGRP = NCH // GRP
N_DVE_S = 6      # groups 0..N_DVE_S-1 compute s on DVE; rest on ACT relu


def _activation_raw(nc, out, in_, func, bias, scale=1.0):
    """nc.scalar.activation minus the Rsqrt accuracy guard.

    Safe here: rsqrt feeds only the soft-threshold scale, where its error is
    attenuated by thr/mag (absolute z error <= eps * thr ~ 1e-6); the final
    output magnitude uses the accurate Sqrt path instead.
    """
    inputs = [nc.scalar.lower_ap(in_)]
    for arg in (bias, scale, 0.0):
        if isinstance(arg, float):
            inputs.append(mybir.ImmediateValue(dtype=F32, value=arg))
        else:
            inputs.append(nc.scalar.lower_ap(arg))
    return nc.scalar.add_instruction(
        mybir.InstActivation(
            name=nc.get_next_instruction_name(),
            func=func,
            ins=inputs,
            outs=[nc.scalar.lower_ap(out)],
        )
    )


def _momentum_scalars():
    """FISTA momentum coefficients with a = fp16(1+gam) and b = 1-a (exact in
    fp16 since a-1 is a multiple of 2^-10 below 1), so the net z-coefficient
    rounding cancels; only the gam*(z - z_old) part sees ~5e-4 rounding."""
    ts_ = [1.0]
    for _ in range(MAX_ITER + 1):
        ts_.append((1.0 + np.sqrt(1.0 + 4.0 * ts_[-1] ** 2)) / 2.0)
    alphas, betas = [], []
    # iteration j computes z_{j+1} from w_j = a_j z_j + b_j z_{j-1},
    # gam_j = (t_{j-1} - 1) / t_j; gam_0 = 0 is realized by the z=0 start.
    for j in range(MAX_ITER):
        gam = 0.0 if j == 0 else (ts_[j - 1] - 1.0) / ts_[j]
        a_hat = float(np.float16(1.0 + gam))
        alphas.append(a_hat)
        betas.append(float(1.0 - a_hat))
    return alphas, betas


def build_nc():
    nc = bacc.Bacc(None)
    W1_d = nc.declare_dram_parameter("W1cat", [P, NCH, P], F32R, isOutput=False)
    W2_d = nc.declare_dram_parameter("W2cat", [P, NCH, P], FP16, isOutput=False)
    AB_d = nc.declare_dram_parameter("AB", [P, 2 * MAX_ITER, P], F32R, isOutput=False)
    Rot_d = nc.declare_dram_parameter("Rotm", [P, P], F32R, isOutput=False)
    Xc_d = nc.declare_dram_parameter("Xcat", [P, 2 * B], F32R, isOutput=False)
    Xc16_d = nc.declare_dram_parameter("Xcat16", [P, 2 * B], FP16, isOutput=False)
    mag_d = nc.declare_dram_parameter("magT", [P, NCH, B], F32, isOutput=True)

    alphas, betas = _momentum_scalars()

    with tile.TileContext(nc) as tc, ExitStack() as ctx:
        state = ctx.enter_context(tc.tile_pool(name="state", bufs=1))
        temps = ctx.enter_context(tc.tile_pool(name="temps", bufs=3))
        psum_u = ctx.enter_context(tc.tile_pool(name="psum_u", bufs=3, space="PSUM"))
        psum_p = ctx.enter_context(tc.tile_pool(name="psum_p", bufs=1, space="PSUM"))

        # ---- persistent SBUF tensors
        W1 = state.tile([P, NCH, P], F32R, tag="W1")
        W2 = state.tile([P, NCH, P], FP16, tag="W2")
        AB = state.tile([P, 2 * MAX_ITER, P], F32R, tag="AB")
        Rotm = state.tile([P, P], F32R, tag="Rotm")
        Xcat = state.tile([P, 2 * B], F32R, tag="Xcat")
        Xcat16 = state.tile([P, 2 * B], FP16, tag="Xcat16")
        zA = state.tile([P, NCH, 2 * B], F32R, tag="zA")
        zB = state.tile([P, NCH, 2 * B], F32R, tag="zB")
        PcatS = state.tile([P, 2 * B], F32R, tag="PcatS")
        rotS = state.tile([P, 2 * B], F32R, tag="rotS")
        Rcat = state.tile([P, 2 * B], FP16, tag="Rcat")
        P2A = state.tile([P, B], F32R, tag="P2A")
        P2B = state.tile([P, B], F32R, tag="P2B")
        R2A = state.tile([P, B], F32R, tag="R2A")
        R2B = state.tile([P, B], F32R, tag="R2B")
        tL = state.tile([P, B], F32R, tag="tL")
        tR = state.tile([P, B], F32R, tag="tR")
        XPbL = state.tile([P, B], F32R, tag="XPbL")
        XPbR = state.tile([P, B], F32R, tag="XPbR")
        magT = state.tile([P, NCH, B], F32, tag="magT")
        zero_col = state.tile([P, 1], F32, tag="zc")
        one_col = state.tile([P, 1], F32, tag="oc")
        eps_col = state.tile([P, 1], F32, tag="ec")

        nc.sync.dma_start(Xcat16[:], Xc16_d[:])
        nc.sync.dma_start(W2[:], W2_d[:])
        nc.sync.dma_start(Xcat[:], Xc_d[:])
        nc.sync.dma_start(W1[:], W1_d[:])
        nc.sync.dma_start(AB[:], AB_d[:])
        nc.sync.dma_start(Rotm[:], Rot_d[:])

        nc.vector.memset(zero_col[:], 0.0)
        nc.vector.memset(one_col[:], 1.0)
        nc.vector.memset(eps_col[:], 1e-30)

        zbuf = [zA, zB]
        P2buf = [P2A, P2B]
        R2buf = [R2A, R2B]
        Pcat_cur = None    # PSUM tile the in-flight A-chain accumulates into
        Pcat_done = None
        pending_A = []     # leftover A-chain groups: (zsrc, group)

        def emit_A_group(zsrc, g, pcat):
            for ci in range(GRP):
                c = GRP * g + ci
                nc.tensor.matmul(
                    pcat[:], W1[:, c, :], zsrc[:, c, :],
                    start=(c == 0), stop=(c == NCH - 1), skip_group_check=True,
                )

        for j in range(MAX_ITER):
            a, b = alphas[j], betas[j]
            first = j == 0
            last = j == MAX_ITER - 1
            have_b = b != 0.0
            aI = AB[:, 2 * j, :]
            bI = AB[:, 2 * j + 1, :]

            z_prev = zbuf[j % 2]       # z_j
            z_new = zbuf[(j + 1) % 2]  # holds z_{j-1}; overwritten -> z_{j+1}
            P2c, P2o = P2buf[j % 2], P2buf[(j + 1) % 2]
            R2c, R2o = R2buf[j % 2], R2buf[(j + 1) % 2]

            grad_src = Xcat16 if first else Rcat
            utiles = {}
            thr_tmp = {}
            # u is accumulated NEGATED (host ships -a/-b identities and -X):
            # Rcat = +SC*(a*P2 + b*P2_old - X) so grads add +step*D^H resid.
            sa = float(np.float32(SC) * np.float32(a))
            sb = float(np.float32(SC) * np.float32(b))

            def emit_mom(g):
                u_ps = psum_u.tile([P, GRP, 2 * B], F32, tag="u")
                utiles[g] = u_ps
                for pi in range(GRP // 2):
                    c2 = GRP * g + 2 * pi
                    out_sl = u_ps[:, 2 * pi:2 * pi + 2, :].rearrange(
                        "p c n -> p (c n)"
                    )
                    nc.tensor.matmul(
                        out_sl, aI,
                        z_prev[:, c2:c2 + 2, :].rearrange("p c n -> p (c n)"),
                        start=True, stop=False, skip_group_check=True,
                    )
                    if have_b:
                        nc.tensor.matmul(
                            out_sl, bI,
                            z_new[:, c2:c2 + 2, :].rearrange("p c n -> p (c n)"),
                            start=False, stop=False, skip_group_check=True,
                        )

            def halves_of(g):
                return ((0, 2), (2, 4)) if g >= 6 else ((0, GRP),)

            def emit_grad_sq(gg):
                if first:
                    u_ps = psum_u.tile([P, GRP, 2 * B], F32, tag="u")
                    utiles[gg] = u_ps
                u_ps = utiles[gg]
                for ci in range(GRP):
                    c = GRP * gg + ci
                    nc.tensor.matmul(
                        u_ps[:, ci, :], W2[:, c, :], grad_src[:],
                        start=(first and ci % 2 == 0),
                        stop=(ci == GRP - 1), skip_group_check=True,
                    )
                t12 = temps.tile([P, GRP, 2 * B], F32, tag="t12")
                m2 = temps.tile([P, GRP, B], F32, tag="m2")
                for h0, h1 in halves_of(gg):
                    nc.scalar.activation(
                        t12[:, h0:h1, :], u_ps[:, h0:h1, :], ACTF.Square,
                        bias=zero_col[:],
                    )
                    nc.gpsimd.tensor_tensor(
                        m2[:, h0:h1, :], t12[:, h0:h1, 0:B],
                        t12[:, h0:h1, B:2 * B], ALU.add,
                    )
                thr_tmp[gg] = (u_ps, m2)

            def emit_tail(gt):
                u_ps, m2 = thr_tmp.pop(gt)
                st = temps.tile([P, GRP, B], F32, tag="st")
                sf = temps.tile([P, GRP, B], F32, tag="srelu")
                mg = None
                if last:
                    mg = temps.tile([P, GRP, B], F32, tag="mag")
                for h0, h1 in halves_of(gt):
                    # st = rsqrt(m2 * 2^26) = thr * rsqrt(m2)
                    _activation_raw(
                        nc, st[:, h0:h1, :], m2[:, h0:h1, :], ACTF.Rsqrt,
                        bias=eps_col[:], scale=RSQ_SCALE,
                    )
                    if last:
                        nc.scalar.activation(
                            sf[:, h0:h1, :], st[:, h0:h1, :], ACTF.Relu,
                            bias=one_col[:], scale=-1.0,
                        )
                    else:
                        # smin = min(st,1) - 1 = -s; z = u_neg * smin = u*s
                        nc.vector.tensor_scalar(
                            sf[:, h0:h1, :], st[:, h0:h1, :], 1.0, 1.0,
                            ALU.min, ALU.subtract,
                        )
                    if not last:
                        z_sl = z_new[:, GRP * gt + h0:GRP * gt + h1, :]
                        z_view = z_sl.rearrange("p c (t b) -> p c t b", t=2)
                        u_view = u_ps[:, h0:h1, :].rearrange(
                            "p c (t b) -> p c t b", t=2
                        )
                        s_b = sf[:, h0:h1, None, :].to_broadcast(
                            [P, h1 - h0, 2, B]
                        )
                        nc.vector.tensor_tensor(z_view, u_view, s_b, ALU.mult)
                    else:
                        nc.scalar.activation(
                            mg[:, h0:h1, :], m2[:, h0:h1, :], ACTF.Sqrt,
                            bias=eps_col[:],
                        )
                        nc.vector.tensor_tensor(
                            magT[:, GRP * gt + h0:GRP * gt + h1, :],
                            mg[:, h0:h1, :], sf[:, h0:h1, :], ALU.mult,
                        )
                        nc.sync.dma_start(
                            mag_d[:, GRP * gt + h0:GRP * gt + h1, :],
                            magT[:, GRP * gt + h0:GRP * gt + h1, :],
                        )

            # ---- prologue: leftover A groups, momentum 0-2, rot + Rcat build
            if not first:
                for _ in range(3):         # A groups 4, 5, 6 of this iter's chain
                    zsrc, gk = pending_A.pop(0)
                    emit_A_group(zsrc, gk, Pcat_cur)
                emit_mom(0)
                emit_mom(1)
                zsrc, gk = pending_A.pop(0)  # group 7
                emit_A_group(zsrc, gk, Pcat_cur)
                Pcat_done = Pcat_cur
                Pcat_cur = None
                # XPb = sb*P2_old + SC*[X | rot(X)]  (P2_old ready long ago)
                if have_b:
                    nc.vector.scalar_tensor_tensor(
                        XPbL[:], P2o[:], sb, Xcat[:, 0:B], ALU.mult, ALU.add
                    )
                    nc.vector.scalar_tensor_tensor(
                        XPbR[:], R2o[:], sb, Xcat[:, B:2 * B], ALU.mult, ALU.add
                    )
                else:
                    nc.vector.tensor_copy(XPbL[:], Xcat[:, 0:B])
                    nc.vector.tensor_copy(XPbR[:], Xcat[:, B:2 * B])
                nc.vector.tensor_copy(PcatS[:], Pcat_done[:])
                rotp = psum_p.tile([P, 2 * B], F32, tag="pcat")
                nc.tensor.matmul(rotp[:], Rotm[:], PcatS[:], start=True, stop=True)
                # v0/v2 overlap the rot matmul (depend only on PcatS)
                nc.vector.scalar_tensor_tensor(
                    tL[:], PcatS[:, 0:B], sa, XPbL[:], ALU.mult, ALU.add
                )
                nc.vector.scalar_tensor_tensor(
                    tR[:], PcatS[:, B:2 * B], sa, XPbR[:], ALU.mult, ALU.add
                )
                nc.vector.tensor_copy(rotS[:], rotp[:])
                # Rcat_L = sa*(PcatL - rotR) + XPbL ; Rcat_R = sa*(rotL + PcatR) + XPbR
                nc.vector.scalar_tensor_tensor(
                    Rcat[:, 0:B], rotS[:, B:2 * B], -sa, tL[:], ALU.mult, ALU.add
                )
                nc.vector.scalar_tensor_tensor(
                    Rcat[:, B:2 * B], rotS[:, 0:B], sa, tR[:], ALU.mult, ALU.add
                )
                emit_mom(2)
                # P2/R2 for the next iteration's XPb (off the critical path)
                nc.gpsimd.tensor_tensor(
                    P2c[:], PcatS[:, 0:B], rotS[:, B:2 * B], ALU.subtract
                )
                nc.gpsimd.tensor_tensor(
                    R2c[:], rotS[:, 0:B], PcatS[:, B:2 * B], ALU.add
                )

            # ---- main group blocks
            for g in range(NGRP):
                if g >= 1:
                    emit_tail(g - 1)
                if not first and g >= 3:
                    emit_mom(g)
                emit_grad_sq(g)
                ka = g - 4
                if not last and ka >= 0:
                    if ka == 0:
                        Pcat_cur = psum_p.tile([P, 2 * B], F32, tag="pcat")
                    emit_A_group(z_new, ka, Pcat_cur)
            emit_tail(NGRP - 1)

            if not last:
                for gk in range(4, NGRP):
                    pending_A.append((z_new, gk))

    nc.finalize()
    return nc


def prep_host_inputs(x, D):
    """Builds per-core input maps from the full inputs."""
    Dr = np.ascontiguousarray(D.real).astype(np.float32)   # [T, F]
    Di = np.ascontiguousarray(D.imag).astype(np.float32)

    # W1cat[k, c, m]: m<64 -> Dr[m, 128c+k]; m>=64 -> Di[m-64, 128c+k]
    W1cat = np.concatenate(
        [Dr.T.reshape(NCH, P, T), Di.T.reshape(NCH, P, T)], axis=2
    ).transpose(1, 0, 2)
    W1cat = np.ascontiguousarray(W1cat).astype(np.float32)

    # W2cat[k, c, m]: k<64 -> Dr[k, 128c+m]; k>=64 -> Di[k-64, 128c+m]
    # scaled by 1/64 (pairs with Rcat's 64*step pre-scale: 64*step/64 = step)
    W2cat = np.concatenate(
        [Dr.reshape(T, NCH, P), Di.reshape(T, NCH, P)], axis=0
    ) * np.float32(1.0 / 64.0)
    W2cat = np.ascontiguousarray(W2cat).astype(np.float16)

    # rot(v) = [v[64:]; -v[:64]]
    Rotm = np.zeros((P, P), dtype=np.float32)
    for m in range(T):
        Rotm[m + T, m] = 1.0
        Rotm[m, m + T] = -1.0

    alphas, betas = _momentum_scalars()
    AB = np.zeros((P, 2 * MAX_ITER, P), dtype=np.float32)
    eye = np.eye(P, dtype=np.float32)
    for j in range(MAX_ITER):
        AB[:, 2 * j, :] = eye * np.float32(-alphas[j])
        AB[:, 2 * j + 1, :] = eye * np.float32(-betas[j])

    in_maps = []
    for i in range(NCORES):
        xs = x[i * B:(i + 1) * B]
        xr = xs[:, 0].astype(np.float32).T   # [T, B]
        xi = xs[:, 1].astype(np.float32).T
        Xcat = np.zeros((P, 2 * B), dtype=np.float32)
        Xcat[0:T, 0:B] = -SC * xr
        Xcat[T:P, 0:B] = -SC * xi
        Xcat[0:T, B:2 * B] = -SC * xi
        Xcat[T:P, B:2 * B] = SC * xr
        in_maps.append({
            "W1cat": W1cat, "W2cat": W2cat, "AB": AB, "Rotm": Rotm,
            "Xcat": Xcat, "Xcat16": Xcat.astype(np.float16),
        })
    return in_maps


def gather_output(results):
    outs = []
    for i in range(NCORES):
        magT = results[i]["magT"].reshape(P, NCH, B)
        outs.append(np.ascontiguousarray(magT.transpose(2, 1, 0)).reshape(B, F))
    mag_all = np.concatenate(outs, axis=0)
    return (mag_all / mag_all.max()).astype(np.float32)


_NC_CACHE = {}


def get_nc():
    if "nc" not in _NC_CACHE:
        _NC_CACHE["nc"] = build_nc()
    return _NC_CACHE["nc"]


def kernel(x, D):
    x = np.asarray(x)
    D = np.asarray(D)
    nc = get_nc()
    in_maps = prep_host_inputs(x, D)
    res = run_bass_kernel_spmd(nc, in_maps, list(range(NCORES)))
    return gather_output(res.results)


if __name__ == "__main__":
    import reference as ref
    inputs = ref.setup_inputs()
    out = kernel(**{k: np.asarray(v) for k, v in inputs.items()})
    print("kernel output", out.shape, out.dtype)


# revision 13
# speedup vs baseline: 1.2306x; 1.2306x over previous
"""Trainium2 Bass kernel for FISTA sparse coding (nn_FISTA_7550552506950).

Strategy (data-parallel over batch, 8 cores x 128 rows; single-f32 state):
- State z kept TRANSPOSED [F=4096, B=128] on-chip as ONE f32 tensor
  [128p, 32 chunks, 256 cols] (real|imag column halves), SBUF-resident for
  all 25 FISTA iterations. HBM traffic is only the initial weight/x load and
  the final magnitude store.
- The z-carrying matmuls (momentum identities, A-chain) run with float32r
  operands (single-pass relaxed fp32; N>=256 keeps full rate): z precision
  dominates the error budget (fp16 z would cost 1.3e-2 vs the 2e-2 gate;
  f32r's ~12-bit ingest gives ~7e-3).
- The gradient matmuls run fully in fp16: the residual quantization is
  benign (sim: 7e-4). Rcat is pre-scaled by 64*step and W2 by 1/64 so all
  significant fp16 values stay in the normal range.
- A-chain: one N=256 matmul per chunk accumulates the quadrant products
  Pcat = [[Dr zr | Dr zi],[Di zr | Di zi]]  ([128, 256] PSUM).
- A fixed +-1 rotation matrix (partition swap t<->t+64 with sign flip,
  rot(v) = [v_hi; -v_lo]) applied by one N=256 matmul turns the quadrants
  into stacked complex products: P2 = [Re; Im](D z) = PcatL - rot(PcatR) and
  rot(P2) = rot(PcatL) + PcatR, from which GPSIMD+DVE build
  Rcat = -64*step*[resid | rot(resid)] with the FISTA momentum folded in by
  linearity (resid = a*P2 + b*P2_old - X; X terms are host-precomputed).
- Gradient: one N=256 fp16 matmul per chunk u[:,c,:] += W2cat_c^T @ Rcat
  yields both -step*Re(D^H resid) and -step*Im(D^H resid) in one go.
- The elementwise momentum u += a*z + b*z_old is folded into the PSUM
  accumulation via scaled-identity f32r matmuls (N=512), with a = fp16(1+gam)
  and b = 1-a exactly representable so the z-coefficient rounding cancels.
- Soft-threshold: t12 = u^2 (ACT), m2 = t12r+t12i (GPSIMD),
  st = rsqrt(m2 * 2^26) = thr*rsqrt(m2) (ACT raw path, thr folded into the
  activation scale - 1/thr^2 is exactly 2^26), then s = 1 - min(st, 1) via
  two DVE tensor_scalar ops (2x port mode) for most groups / ACT relu for
  the tail groups, z = u*s (DVE, one f32 pass).
- Software pipeline: grad lags momentum by 1 slot, the threshold tail by 2
  (PSUM u pool has 3 bufs = exactly the in-flight window), and the next
  iteration's A-chain follows each group's z-write by 1 slot. The leftover
  A-group + rotation + Rcat build at each iteration boundary hide behind the
  momentum matmuls of the first two slots.
- Global max normalization happens on host during the gather (tiny).
"""

import numpy as np
from contextlib import ExitStack

import concourse.bass as bass
import concourse.mybir as mybir
import concourse.tile as tile
from concourse import bacc
from concourse.bass_utils import run_bass_kernel_spmd

F32 = mybir.dt.float32
F32R = mybir.dt.float32r
FP16 = mybir.dt.float16
BF16 = mybir.dt.bfloat16
ALU = mybir.AluOpType
ACTF = mybir.ActivationFunctionType

P = 128          # partitions / f-chunk size
F = 4096         # dictionary size
T = 64           # signal dim
NCH = F // P     # 32 chunks
B = 128          # batch rows per core
NCORES = 8
MAX_ITER = 25
STEP = np.float32(1.0 / F)
THR = np.float32(0.5) * STEP
SC = np.float32(64.0) * STEP   # fp16 scaling for the gradient path (2^-6)
RSQ_SCALE = float(1.0 / (THR * THR))   # exactly 2^26
GRP = 4          # chunks per elementwise group
NGRP = NCH // GRP
N_DVE_S = 6      # groups 0..N_DVE_S-1 compute s on DVE; rest on ACT relu


def _activation_raw(nc, out, in_, func, bias, scale=1.0):
    """nc.scalar.activation minus the Rsqrt accuracy guard.

    Safe here: rsqrt feeds only the soft-threshold scale, where its error is
    attenuated by thr/mag (absolute z error <= eps * thr ~ 1e-6); the final
    output magnitude uses the accurate Sqrt path instead.
    """
    inputs = [nc.scalar.lower_ap(in_)]
    for arg in (bias, scale, 0.0):
        if isinstance(arg, float):
            inputs.append(mybir.ImmediateValue(dtype=F32, value=arg))
        else:
            inputs.append(nc.scalar.lower_ap(arg))
    return nc.scalar.add_instruction(
        mybir.InstActivation(
            name=nc.get_next_instruction_name(),
            func=func,
            ins=inputs,
            outs=[nc.scalar.lower_ap(out)],
        )
    )


def _momentum_scalars():
    """FISTA momentum coefficients with a = fp16(1+gam) and b = 1-a (exact in
    fp16 since a-1 is a multiple of 2^-10 below 1), so the net z-coefficient
    rounding cancels; only the gam*(z - z_old) part sees ~5e-4 rounding."""
    ts_ = [1.0]
    for _ in range(MAX_ITER + 1):
        ts_.append((1.0 + np.sqrt(1.0 + 4.0 * ts_[-1] ** 2)) / 2.0)
    alphas, betas = [], []
    # iteration j computes z_{j+1} from w_j = a_j z_j + b_j z_{j-1},
    # gam_j = (t_{j-1} - 1) / t_j; gam_0 = 0 is realized by the z=0 start.
    for j in range(MAX_ITER):
        gam = 0.0 if j == 0 else (ts_[j - 1] - 1.0) / ts_[j]
        a_hat = float(np.float16(1.0 + gam))
        alphas.append(a_hat)
        betas.append(float(1.0 - a_hat))
    return alphas, betas


def build_nc():
    nc = bacc.Bacc(None)
    W1_d = nc.declare_dram_parameter("W1cat", [P, NCH, P], F32R, isOutput=False)
    W2_d = nc.declare_dram_parameter("W2cat", [P, NCH, P], FP16, isOutput=False)
    AB_d = nc.declare_dram_parameter("AB", [P, 2 * MAX_ITER, P], F32R, isOutput=False)
    Rot_d = nc.declare_dram_parameter("Rotm", [P, P], F32R, isOutput=False)
    Xc_d = nc.declare_dram_parameter("Xcat", [P, 2 * B], F32R, isOutput=False)
    Xc16_d = nc.declare_dram_parameter("Xcat16", [P, 2 * B], FP16, isOutput=False)
    mag_d = nc.declare_dram_parameter("magT", [P, NCH, B], F32, isOutput=True)

    alphas, betas = _momentum_scalars()

    with tile.TileContext(nc) as tc, ExitStack() as ctx:
        state = ctx.enter_context(tc.tile_pool(name="state", bufs=1))
        temps = ctx.enter_context(tc.tile_pool(name="temps", bufs=3))
        psum_u = ctx.enter_context(tc.tile_pool(name="psum_u", bufs=3, space="PSUM"))
        psum_p = ctx.enter_context(tc.tile_pool(name="psum_p", bufs=1, space="PSUM"))

        # ---- persistent SBUF tensors
        W1 = state.tile([P, NCH, P], F32R, tag="W1")
        W2 = state.tile([P, NCH, P], FP16, tag="W2")
        AB = state.tile([P, 2 * MAX_ITER, P], F32R, tag="AB")
        Rotm = state.tile([P, P], F32R, tag="Rotm")
        Xcat = state.tile([P, 2 * B], F32R, tag="Xcat")
        Xcat16 = state.tile([P, 2 * B], FP16, tag="Xcat16")
        zA = state.tile([P, NCH, 2 * B], F32R, tag="zA")
        zB = state.tile([P, NCH, 2 * B], F32R, tag="zB")
        PcatS = state.tile([P, 2 * B], F32R, tag="PcatS")
        rotS = state.tile([P, 2 * B], F32R, tag="rotS")
        RcatA = state.tile([P, 2 * B], FP16, tag="RcatA")
        RcatB = state.tile([P, 2 * B], FP16, tag="RcatB")
        P2A = state.tile([P, B], F32R, tag="P2A")
        P2B = state.tile([P, B], F32R, tag="P2B")
        R2A = state.tile([P, B], F32R, tag="R2A")
        R2B = state.tile([P, B], F32R, tag="R2B")
        tL = state.tile([P, B], F32R, tag="tL")
        tR = state.tile([P, B], F32R, tag="tR")
        XPbL = state.tile([P, B], F32R, tag="XPbL")
        XPbR = state.tile([P, B], F32R, tag="XPbR")
        magT = state.tile([P, NCH, B], F32, tag="magT")
        zero_col = state.tile([P, 1], F32, tag="zc")
        one_col = state.tile([P, 1], F32, tag="oc")
        eps_col = state.tile([P, 1], F32, tag="ec")
        nthr_col = state.tile([P, 1], F32, tag="ntc")

        nc.sync.dma_start(Xcat16[:], Xc16_d[:])
        nc.sync.dma_start(W2[:], W2_d[:])
        nc.sync.dma_start(Xcat[:], Xc_d[:])
        nc.sync.dma_start(W1[:], W1_d[:])
        nc.sync.dma_start(AB[:], AB_d[:])
        nc.sync.dma_start(Rotm[:], Rot_d[:])

        nc.vector.memset(zero_col[:], 0.0)
        nc.vector.memset(one_col[:], 1.0)
        nc.vector.memset(eps_col[:], 1e-30)
        nc.vector.memset(nthr_col[:], -float(THR))

        zbuf = [zA, zB]
        P2buf = [P2A, P2B]
        R2buf = [R2A, R2B]
        Pcat_cur = None    # PSUM tile the in-flight A-chain accumulates into
        Pcat_done = None
        pending_A = []     # leftover A-chain groups: (zsrc, group)

        def emit_A_group(zsrc, g, pcat):
            for ci in range(GRP):
                c = GRP * g + ci
                nc.tensor.matmul(
                    pcat[:], W1[:, c, :], zsrc[:, c, :],
                    start=(c == 0), stop=(c == NCH - 1), skip_group_check=True,
                )

        for j in range(MAX_ITER):
            a, b = alphas[j], betas[j]
            first = j == 0
            last = j == MAX_ITER - 1
            have_b = b != 0.0
            aI = AB[:, 2 * j, :]
            bI = AB[:, 2 * j + 1, :]

            z_prev = zbuf[j % 2]       # z_j
            z_new = zbuf[(j + 1) % 2]  # holds z_{j-1}; overwritten -> z_{j+1}
            P2c, P2o = P2buf[j % 2], P2buf[(j + 1) % 2]
            R2c, R2o = R2buf[j % 2], R2buf[(j + 1) % 2]

            Rcat = [RcatA, RcatB][j % 2]
            grad_src = Xcat16 if first else Rcat
            utiles = {}
            thr_tmp = {}
            # u is accumulated NEGATED (host ships -a/-b identities and -X):
            # Rcat = +SC*(a*P2 + b*P2_old - X) so grads add +step*D^H resid.
            sa = float(np.float32(SC) * np.float32(a))
            sb = float(np.float32(SC) * np.float32(b))

            def emit_mom(g):
                u_ps = psum_u.tile([P, GRP, 2 * B], F32, tag="u")
                utiles[g] = u_ps
                for pi in range(GRP // 2):
                    c2 = GRP * g + 2 * pi
                    out_sl = u_ps[:, 2 * pi:2 * pi + 2, :].rearrange(
                        "p c n -> p (c n)"
                    )
                    nc.tensor.matmul(
                        out_sl, aI,
                        z_prev[:, c2:c2 + 2, :].rearrange("p c n -> p (c n)"),
                        start=True, stop=False, skip_group_check=True,
                    )
                    if have_b:
                        nc.tensor.matmul(
                            out_sl, bI,
                            z_new[:, c2:c2 + 2, :].rearrange("p c n -> p (c n)"),
                            start=False, stop=False, skip_group_check=True,
                        )

            def halves_of(g):
                return ((0, 2), (2, 4)) if g >= 6 else ((0, GRP),)

            def emit_grad_sq(gg):
                if first:
                    u_ps = psum_u.tile([P, GRP, 2 * B], F32, tag="u")
                    utiles[gg] = u_ps
                u_ps = utiles[gg]
                for ci in range(GRP):
                    c = GRP * gg + ci
                    nc.tensor.matmul(
                        u_ps[:, ci, :], W2[:, c, :], grad_src[:],
                        start=(first and ci % 2 == 0),
                        stop=(ci == GRP - 1), skip_group_check=True,
                    )
                dt_sq = F32 if last else BF16
                t12 = temps.tile([P, GRP, 2 * B], dt_sq, tag="t12")
                m2 = temps.tile([P, GRP, B], dt_sq, tag="m2")
                for h0, h1 in halves_of(gg):
                    nc.scalar.activation(
                        t12[:, h0:h1, :], u_ps[:, h0:h1, :], ACTF.Square,
                        bias=zero_col[:],
                    )
                    nc.gpsimd.tensor_tensor(
                        m2[:, h0:h1, :], t12[:, h0:h1, 0:B],
                        t12[:, h0:h1, B:2 * B], ALU.add,
                    )
                thr_tmp[gg] = (u_ps, m2)

            def emit_tail(gt):
                u_ps, m2 = thr_tmp.pop(gt)
                if last:
                    # mag = |z| = relu(sqrt(m2) - thr): no rsqrt/zmult needed
                    mg = temps.tile([P, GRP, B], F32, tag="mag")
                    nc.scalar.activation(
                        mg[:], m2[:], ACTF.Sqrt, bias=eps_col[:]
                    )
                    nc.scalar.activation(
                        magT[:, GRP * gt:GRP * (gt + 1), :], mg[:], ACTF.Relu,
                        bias=nthr_col[:],
                    )
                    nc.sync.dma_start(
                        mag_d[:, GRP * gt:GRP * (gt + 1), :],
                        magT[:, GRP * gt:GRP * (gt + 1), :],
                    )
                    return
                st = temps.tile([P, GRP, B], F32, tag="st")
                sf = temps.tile([P, GRP, B], F32, tag="srelu")
                for h0, h1 in halves_of(gt):
                    # st = rsqrt(m2 * 2^26) = thr * rsqrt(m2)
                    _activation_raw(
                        nc, st[:, h0:h1, :], m2[:, h0:h1, :], ACTF.Rsqrt,
                        bias=eps_col[:], scale=RSQ_SCALE,
                    )
                    # smin = min(st,1) - 1 = -s; z = u_neg * smin = u*s
                    nc.vector.tensor_scalar(
                        sf[:, h0:h1, :], st[:, h0:h1, :], 1.0, 1.0,
                        ALU.min, ALU.subtract,
                    )
                    c0, c1 = GRP * gt + h0, GRP * gt + h1
                    nc.vector.tensor_tensor(
                        z_new[:, c0:c1, 0:B], u_ps[:, h0:h1, 0:B],
                        sf[:, h0:h1, :], ALU.mult,
                    )
                    nc.vector.tensor_tensor(
                        z_new[:, c0:c1, B:2 * B], u_ps[:, h0:h1, B:2 * B],
                        sf[:, h0:h1, :], ALU.mult,
                    )

            # ---- prologue: leftover A groups, momentum 0-2, rot + Rcat build
            if not first:
                for _ in range(3):         # A groups 4, 5, 6 of this iter's chain
                    zsrc, gk = pending_A.pop(0)
                    emit_A_group(zsrc, gk, Pcat_cur)
                emit_mom(0)
                emit_mom(1)
                zsrc, gk = pending_A.pop(0)  # group 7
                emit_A_group(zsrc, gk, Pcat_cur)
                Pcat_done = Pcat_cur
                Pcat_cur = None
                # XPb = sb*P2_old + SC*[X | rot(X)]  (P2_old ready long ago)
                if have_b:
                    nc.vector.scalar_tensor_tensor(
                        XPbL[:], P2o[:], sb, Xcat[:, 0:B], ALU.mult, ALU.add
                    )
                    nc.vector.scalar_tensor_tensor(
                        XPbR[:], R2o[:], sb, Xcat[:, B:2 * B], ALU.mult, ALU.add
                    )
                else:
                    nc.vector.tensor_copy(XPbL[:], Xcat[:, 0:B])
                    nc.vector.tensor_copy(XPbR[:], Xcat[:, B:2 * B])
                nc.vector.tensor_copy(PcatS[:], Pcat_done[:])
                rotp = psum_p.tile([P, 2 * B], F32, tag="pcat")
                nc.tensor.matmul(rotp[:], Rotm[:], PcatS[:], start=True, stop=True)
                # v0/v2 overlap the rot matmul (depend only on PcatS)
                nc.vector.scalar_tensor_tensor(
                    tL[:], PcatS[:, 0:B], sa, XPbL[:], ALU.mult, ALU.add
                )
                nc.vector.scalar_tensor_tensor(
                    tR[:], PcatS[:, B:2 * B], sa, XPbR[:], ALU.mult, ALU.add
                )
                nc.vector.tensor_copy(rotS[:], rotp[:])
                # Rcat_L = sa*(PcatL - rotR) + XPbL ; Rcat_R = sa*(rotL + PcatR) + XPbR
                nc.vector.scalar_tensor_tensor(
                    Rcat[:, 0:B], rotS[:, B:2 * B], -sa, tL[:], ALU.mult, ALU.add
                )
                nc.vector.scalar_tensor_tensor(
                    Rcat[:, B:2 * B], rotS[:, 0:B], sa, tR[:], ALU.mult, ALU.add
                )
                emit_mom(2)
                # P2/R2 for the next iteration's XPb (off the critical path)
                nc.gpsimd.tensor_tensor(
                    P2c[:], PcatS[:, 0:B], rotS[:, B:2 * B], ALU.subtract
                )
                nc.gpsimd.tensor_tensor(
                    R2c[:], rotS[:, 0:B], PcatS[:, B:2 * B], ALU.add
                )

            # ---- main group blocks
            for g in range(NGRP):
                if g >= 1:
                    emit_tail(g - 1)
                if not first and g >= 3:
                    emit_mom(g)
                emit_grad_sq(g)
                ka = g - 4
                if not last and ka >= 0:
                    if ka == 0:
                        Pcat_cur = psum_p.tile([P, 2 * B], F32, tag="pcat")
                    emit_A_group(z_new, ka, Pcat_cur)
            emit_tail(NGRP - 1)

            if not last:
                for gk in range(4, NGRP):
                    pending_A.append((z_new, gk))

    nc.finalize()
    return nc


def prep_host_inputs(x, D):
    """Builds per-core input maps from the full inputs."""
    Dr = np.ascontiguousarray(D.real).astype(np.float32)   # [T, F]
    Di = np.ascontiguousarray(D.imag).astype(np.float32)

    # W1cat[k, c, m]: m<64 -> Dr[m, 128c+k]; m>=64 -> Di[m-64, 128c+k]
    W1cat = np.concatenate(
        [Dr.T.reshape(NCH, P, T), Di.T.reshape(NCH, P, T)], axis=2
    ).transpose(1, 0, 2)
    W1cat = np.ascontiguousarray(W1cat).astype(np.float32)

    # W2cat[k, c, m]: k<64 -> Dr[k, 128c+m]; k>=64 -> Di[k-64, 128c+m]
    # scaled by 1/64 (pairs with Rcat's 64*step pre-scale: 64*step/64 = step)
    W2cat = np.concatenate(
        [Dr.reshape(T, NCH, P), Di.reshape(T, NCH, P)], axis=0
    ) * np.float32(1.0 / 64.0)
    W2cat = np.ascontiguousarray(W2cat).astype(np.float16)

    # rot(v) = [v[64:]; -v[:64]]
    Rotm = np.zeros((P, P), dtype=np.float32)
    for m in range(T):
        Rotm[m + T, m] = 1.0
        Rotm[m, m + T] = -1.0

    alphas, betas = _momentum_scalars()
    AB = np.zeros((P, 2 * MAX_ITER, P), dtype=np.float32)
    eye = np.eye(P, dtype=np.float32)
    for j in range(MAX_ITER):
        AB[:, 2 * j, :] = eye * np.float32(-alphas[j])
        AB[:, 2 * j + 1, :] = eye * np.float32(-betas[j])

    in_maps = []
    for i in range(NCORES):
        xs = x[i * B:(i + 1) * B]
        xr = xs[:, 0].astype(np.float32).T   # [T, B]
        xi = xs[:, 1].astype(np.float32).T
        Xcat = np.zeros((P, 2 * B), dtype=np.float32)
        Xcat[0:T, 0:B] = -SC * xr
        Xcat[T:P, 0:B] = -SC * xi
        Xcat[0:T, B:2 * B] = -SC * xi
        Xcat[T:P, B:2 * B] = SC * xr
        in_maps.append({
            "W1cat": W1cat, "W2cat": W2cat, "AB": AB, "Rotm": Rotm,
            "Xcat": Xcat, "Xcat16": Xcat.astype(np.float16),
        })
    return in_maps


def gather_output(results):
    outs = []
    for i in range(NCORES):
        magT = results[i]["magT"].reshape(P, NCH, B)
        outs.append(np.ascontiguousarray(magT.transpose(2, 1, 0)).reshape(B, F))
    mag_all = np.concatenate(outs, axis=0)
    return (mag_all / mag_all.max()).astype(np.float32)


_NC_CACHE = {}


def get_nc():
    if "nc" not in _NC_CACHE:
        _NC_CACHE["nc"] = build_nc()
    return _NC_CACHE["nc"]


def kernel(x, D):
    x = np.asarray(x)
    D = np.asarray(D)
    nc = get_nc()
    in_maps = prep_host_inputs(x, D)
    res = run_bass_kernel_spmd(nc, in_maps, list(range(NCORES)))
    return gather_output(res.results)


if __name__ == "__main__":
    import reference as ref
    inputs = ref.setup_inputs()
    out = kernel(**{k: np.asarray(v) for k, v in inputs.items()})
    print("kernel output", out.shape, out.dtype)
